# revision 1
# baseline (speedup 1.0000x reference)
"""Trainium2 Bass kernel for nn_KSpaceLoss: exact type-2 NUFFT k-space loss.

loss = 0.1 * (sum|d| / sum|a|) + 0.1 * sqrt(sum d^2 / sum a^2)
  d = (E @ x) * mask - kdata * mask,  a = kdata * mask
  E[k, n] = exp(-2j*pi * traj[:, k] . r[:, n])   (K=8192, N=96*96)

Sharding: K axis split across 8 NeuronCores (1024 samples each).

Structure (per core):
 - Mirror pairing: E(-r) = conj(E(r)); host pairs grid points r/-r, so only
   4704 representative points (37 chunks of 128, padded) need phase/trig.
   Paired contribution with u± = xr ± xr', v± = xi ± xi':
     Re += er*u+ + ei*(-v-) ;  Im += er*v+ + ei*u-
 - PE: ph = 4-row split-precision phase matmul (bf16 hi/lo traj, fp32 PSUM)
 - rnd = fl(ph+MAGIC): split ACT (Identity + MAGIC bias) / VE (ts add),
   pattern c%3==2 or c==0 (empirically tuned; one PSUM operand per VE op
   forces rnd through SBUF before the stt)
 - VE: mf = (rnd - MAGIC) - ph = -frac(phase) (f16); uu = |mf| (bitand,
   supertile-wide; per-chunk on the first supertile to shorten fill)
 - ACT: er = sin(pi/2 - 2pi*uu) = cos(2pi*ph); ei = sin(2pi*mf), one
   2048-col Sin each per 2-chunk supertile
 - PE: fp8(e4m3) DoubleRow matmuls (2 chunks/stream, 0.5 cyc/row):
   ps[0:64, k] accumulates er-stream @ [u+;v+] and ei-stream @ [-v-;u-]
 - software pipeline: phase+rangereduce(t) / sins(t-1) / matmuls(t-2);
   within a supertile: all phase matmuls, then rnds, then stts
 - ps transits PSUM->SBUF per k-half and DMAs out (sync queue only);
   mask/residual/norms + weighted combine run on the host in float64.
"""

import math

import numpy as np
import ml_dtypes

import concourse.bacc as bacc
import concourse.tile as tile
from concourse import mybir
from concourse.bass_utils import run_bass_kernel_spmd

X, Y, Z = 96, 96, 1
C, S, T = 8, 1, 4
K = 8192
N = X * Y * Z
NCORES = 8
KL = K // NCORES          # 1024 k-samples per core
NR = 4736                 # padded representative points (37 chunks)
NCH = NR // 128           # 37
NCHW = 38                 # weight chunks (padded even for DR pairing)
SC = 2                    # chunks per sin supertile
CST = C * S * T           # 32
W1, W2 = 0.1, 0.1

F32 = mybir.dt.float32
F8 = mybir.dt.float8e4
U16 = mybir.dt.uint16
F16 = mybir.dt.float16
BF16 = mybir.dt.bfloat16
PI = math.pi
MAGIC = 12582912.0          # 1.5 * 2^23: fl(x + MAGIC) - MAGIC == round(x)


def build_kernel():
    nc = bacc.Bacc("TRN2", target_bir_lowering=False, debug=False,
                   num_devices=NCORES)

    w1_d = nc.dram_tensor("w1", [128, NCHW, 64], F8, kind="ExternalInput").ap()
    w2_d = nc.dram_tensor("w2", [128, NCHW, 64], F8, kind="ExternalInput").ap()
    r2_d = nc.dram_tensor("r2", [4, NR], BF16, kind="ExternalInput").ap()
    tw_d = nc.dram_tensor("tw", [4, KL], BF16, kind="ExternalInput").ap()
    pso_d = nc.dram_tensor("pso", [64, KL], F32, kind="ExternalOutput").ap()

    Sin = mybir.ActivationFunctionType.Sin
    Sqrt = mybir.ActivationFunctionType.Sqrt
    Ident = mybir.ActivationFunctionType.Identity
    Alu = mybir.AluOpType

    with tile.TileContext(nc) as tc:
        with (
            tc.tile_pool(name="const", bufs=1) as cpool,
            tc.tile_pool(name="ph", bufs=3, space="PSUM") as php,
            tc.tile_pool(name="acc", bufs=1, space="PSUM") as accp,
            tc.tile_pool(name="rnd", bufs=4) as rnp,
            tc.tile_pool(name="mwork", bufs=4) as vwp,
            tc.tile_pool(name="ework", bufs=6) as ewp,
            tc.tile_pool(name="resid", bufs=1) as rsp,
        ):
            r2 = cpool.tile([4, NR], BF16, tag="r2")
            tw = cpool.tile([4, KL], BF16, tag="tw")
            w1 = cpool.tile([128, NCHW, 64], F8, tag="w1")
            w2 = cpool.tile([128, NCHW, 64], F8, tag="w2")
            nc.sync.dma_start(r2[:], r2_d[:])
            nc.gpsimd.dma_start(tw[:], tw_d[:])
            nc.gpsimd.dma_start(w1[:], w1_d[:])
            nc.gpsimd.dma_start(w2[:], w2_d[:])

            bias_cos = cpool.tile([128, 1], F32, tag="bcos")
            nc.vector.memset(bias_cos[:], PI / 2)
            bias_magic = cpool.tile([128, 1], F32, tag="bmag")
            nc.vector.memset(bias_magic[:], MAGIC)

            ps = accp.tile([64, KL], F32, tag="ps")

            # supertiles: sins batched over `sc` chunks each
            SUPS = []
            c0 = 0
            while c0 < NCH:
                sc = min(2, NCH - c0)
                SUPS.append((c0, sc))
                c0 += sc
            NSUP = len(SUPS)
            DR = mybir.MatmulPerfMode.DoubleRow
            etiles = {}

            vtiles = {}

            def emit_pre(s):
                c0, sc = SUPS[s]
                # planes: [0,1] = |m| (-> er), [2,3] = m (-> ei)
                vf = vwp.tile([128, 4, KL], F16, tag="vf")
                vtiles[s] = vf
                if sc < 2:
                    nc.vector.memset(vf[:, 1, :], 0.0)
                    nc.vector.memset(vf[:, 3, :], 0.0)
                phs, rnds = [], []
                for h in range(sc):
                    c = c0 + h
                    lhs4 = r2[0:4, c * 128:(c + 1) * 128]
                    ph = php.tile([128, KL], F32, tag="ph")
                    for q in range(2):
                        sl = slice(q * 512, (q + 1) * 512)
                        nc.tensor.matmul(ph[:, sl], lhs4, tw[0:4, sl],
                                         start=True, stop=True)
                    phs.append(ph)
                for h in range(sc):
                    c = c0 + h
                    rndS = rnp.tile([128, KL], F32, tag="rnd")
                    if c % 3 == 2 or c == 0:
                        nc.scalar.activation(rndS[:], phs[h][:], Ident,
                                             bias=bias_magic[:], scale=1.0)
                    else:
                        nc.vector.tensor_scalar(rndS[:], phs[h][:], MAGIC,
                                                None, op0=Alu.add)
                    rnds.append(rndS)
                if s == 0:
                    # fill path: interleave uu per chunk so the first er-sin
                    # dependency completes as early as possible
                    for h in range(sc):
                        nc.vector.scalar_tensor_tensor(
                            vf[:, 2 + h, :], rnds[h][:], MAGIC, phs[h][:],
                            op0=Alu.subtract, op1=Alu.subtract)
                        nc.vector.tensor_scalar(
                            vf[:, h, :].bitcast(U16),
                            vf[:, 2 + h, :].bitcast(U16),
                            0x7FFF, None, op0=Alu.bitwise_and)
                else:
                    for h in range(sc):
                        nc.vector.scalar_tensor_tensor(
                            vf[:, 2 + h, :], rnds[h][:], MAGIC, phs[h][:],
                            op0=Alu.subtract, op1=Alu.subtract)
                    nc.vector.tensor_scalar(
                        vf[:, 0:2, :].bitcast(U16),
                        vf[:, 2:4, :].bitcast(U16),
                        0x7FFF, None, op0=Alu.bitwise_and)

            def emit_sins(s):
                c0, sc = SUPS[s]
                vf = vtiles.pop(s)
                ee = ewp.tile([128, 4, KL], F8, tag="ee")
                etiles[s] = ee
                nc.scalar.activation(ee[:, 2:4, :], vf[:, 2:4, :], Sin,
                                     bias=0.0, scale=2 * PI)
                nc.scalar.activation(ee[:, 0:2, :], vf[:, 0:2, :], Sin,
                                     bias=bias_cos[:], scale=-2 * PI)

            def emit_back(s):
                c0, sc = SUPS[s]
                ee = etiles.pop(s)
                first = s == 0
                last = s == NSUP - 1
                wsl = slice(c0, c0 + 2)
                for j in range(2):
                    sl = slice(j * 512, (j + 1) * 512)
                    nc.tensor.matmul(ps[:, sl], w1[:, wsl, :],
                                     ee[:, 0:2, sl],
                                     start=first, stop=False, perf_mode=DR)
                for j in range(2):
                    sl = slice(j * 512, (j + 1) * 512)
                    nc.tensor.matmul(ps[:, sl], w2[:, wsl, :],
                                     ee[:, 2:4, sl],
                                     start=False, stop=last, perf_mode=DR)

            PF = 1
            for t in range(NSUP + 1 + PF):
                if t < NSUP:
                    emit_pre(t)
                if 1 <= t <= NSUP:
                    emit_sins(t - 1)
                if t >= 1 + PF:
                    emit_back(t - 1 - PF)

            # residual moved to host: just transit ps PSUM->SBUF and DMA
            psS = rsp.tile([64, KL], F32, tag="psS")
            for j in range(2):
                sl = slice(j * 512, (j + 1) * 512)
                nc.vector.tensor_scalar(psS[:, sl], ps[:, sl], 0.0, None,
                                        op0=Alu.add)
                nc.sync.dma_start(pso_d[:, sl], psS[:, sl])

    nc.compile()
    return nc


_NC_CACHE = []


def _get_nc():
    if not _NC_CACHE:
        _NC_CACHE.append(build_kernel())
    return _NC_CACHE[0]


def _host_prep(images_reconstructed, kspace_trajectory, kspace_data,
               kspace_mask, sensitivity_maps):
    img = np.asarray(images_reconstructed)
    traj = np.asarray(kspace_trajectory).astype(np.float32)
    kdata = np.asarray(kspace_data)
    mask = np.asarray(kspace_mask).astype(np.float32)
    smaps = np.asarray(sensitivity_maps)
    bf = ml_dtypes.bfloat16

    x = 0.5 * img[None, ...] * smaps[..., None, None]      # (C,X,Y,Z,S,T)
    xw = x.reshape(C, N, T).transpose(1, 0, 2).reshape(N, CST)

    # mirror pairing: E(-r) = conj(E(r))
    GX, GY = np.meshgrid(np.arange(X) - 48, np.arange(Y) - 48, indexing="ij")
    gxf, gyf = GX.ravel(), GY.ravel()
    n_arr = np.arange(N)
    has_m = (gxf >= -47) & (gyf >= -47)
    mirror_n = np.where(has_m, (48 - gxf) * 96 + (48 - gyf), -1)
    is_rep = (~has_m) | (n_arr <= mirror_n)
    idx = n_arr[is_rep]
    midx = mirror_n[is_rep]
    midx = np.where(midx == idx, -1, midx)
    pad = NR - len(idx)

    xr = xw.real.astype(np.float32)
    xi = xw.imag.astype(np.float32)
    sel = np.maximum(midx, 0)
    on = (midx[:, None] >= 0)
    xr_m = np.where(on, xr[sel], 0.0)
    xi_m = np.where(on, xi[sel], 0.0)
    w1 = np.concatenate([xr[idx] + xr_m, xi[idx] + xi_m], 1)   # [u+; v+]
    w2 = np.concatenate([-(xi[idx] - xi_m), xr[idx] - xr_m], 1)  # [-v-; u-]
    wpad = NCHW * 128 - len(idx)
    zpad = np.zeros((wpad, 64), np.float32)
    f8 = ml_dtypes.float8_e4m3
    w1 = np.ascontiguousarray(np.vstack([w1, zpad]).astype(f8)
                              .reshape(NCHW, 128, 64).transpose(1, 0, 2))
    w2 = np.ascontiguousarray(np.vstack([w2, zpad]).astype(f8)
                              .reshape(NCHW, 128, 64).transpose(1, 0, 2))

    gxr = np.concatenate([gxf[is_rep], np.zeros(pad)]).astype(np.float32)
    gyr = np.concatenate([gyf[is_rep], np.zeros(pad)]).astype(np.float32)
    r2 = np.stack([gxr, gxr, gyr, gyr]).astype(bf)

    t2 = traj[:2]
    th = t2.astype(bf)
    tl = (t2 - th.astype(np.float32)).astype(bf)
    tw5 = np.stack([th[0], tl[0], th[1], tl[1]])

    mk = mask.reshape(K).astype(np.float32)
    kd = kdata.reshape(C, K, T).transpose(1, 0, 2).reshape(K, CST)
    kdm = kd * mk[:, None]

    in_maps = []
    for i in range(NCORES):
        ksl = slice(i * KL, (i + 1) * KL)
        in_maps.append({
            "w1": w1, "w2": w2, "r2": r2,
            "tw": np.ascontiguousarray(tw5[:, ksl]),
        })
    return in_maps, kdm, mk


def kernel(images_reconstructed, kspace_trajectory, kspace_data,
           kspace_mask, sensitivity_maps, _trace=False):
    nc = _get_nc()
    in_maps, kdm, mk = _host_prep(images_reconstructed, kspace_trajectory,
                                  kspace_data, kspace_mask, sensitivity_maps)
    kw = {"tmpdir": "/tmp/bass_trace"} if _trace else {}
    res = run_bass_kernel_spmd(nc, in_maps, core_ids=list(range(NCORES)),
                               trace=_trace, **kw)
    pso = np.concatenate([res.results[i]["pso"] for i in range(NCORES)],
                         axis=1)                       # (64, K)
    ksp = (pso[:CST] + 1j * pso[CST:]).T.astype(np.complex128)  # (K, CST)
    d = ksp * mk[:, None] - kdm
    ad = np.abs(d)
    l1, l2 = ad.sum(), (ad * ad).sum()
    a = np.abs(kdm)
    a1, a2 = a.sum(), (a * a).sum()
    loss = np.asarray(W1 * (l1 / a1) + W2 * math.sqrt(l2) / math.sqrt(a2),
                      dtype=np.float32)
    if _trace:
        return loss, res
    return loss



# revision 2
# speedup vs baseline: 1.1605x; 1.1605x over previous
"""Trainium2 Bass kernel for nn_KSpaceLoss: exact type-2 NUFFT k-space loss.

v4: uint15 wrapped-phase chain, host-computed double-step deltas,
direct groups front-loaded, 4-chunk sin groups.

Math identical to v2/v3 (see kernel_v2 docstring). Pipeline:
 - groups: [seed(0,1), direct(36,37), direct(34,35)] emitted first (their
   inputs come straight from host DMAs), then 8 chained 4-chunk groups
   (chunks 2..33). PSUM accumulation: start on seed (first emitted),
   stop on the last chain group; transit+output DMA follow immediately.
 - chain: v[c] = (v[c-2] + dd2[(c-2)%3]) & 0x7FFF with host-computed
   rail-duplicated dd2 tiles; the four rail-pairs of a group update
   independently (two from the previous group, two intra) so the critical
   path per group is two VE hops while ACT runs one 4928-elem Sin.
"""

import math

import numpy as np
import ml_dtypes

import concourse.bacc as bacc
import concourse.tile as tile
from concourse import mybir
from concourse.bass_utils import run_bass_kernel_spmd

X, Y, Z = 96, 96, 1
C, S, T = 8, 1, 4
K = 8192
N = X * Y * Z
NCORES = 8
CST = C * S * T
W1, W2 = 0.1, 0.1

NCH = 37
NCHW = 38

F32 = mybir.dt.float32
F8 = mybir.dt.float8e4
U16 = mybir.dt.uint16
PI = math.pi
VSCALE = 32768.0


def build_kernel(kle):
    nc = bacc.Bacc("TRN2", target_bir_lowering=False, debug=False,
                   num_devices=NCORES)

    w1_d = nc.dram_tensor("w1", [128, NCHW, 64], F8, kind="ExternalInput").ap()
    w2_d = nc.dram_tensor("w2", [128, NCHW, 64], F8, kind="ExternalInput").ap()
    seed_d = nc.dram_tensor("seed", [128, 4, kle], U16, kind="ExternalInput").ap()
    dd2_d = nc.dram_tensor("dd2", [128, 3, 2, kle], U16, kind="ExternalInput").ap()
    d34_d = nc.dram_tensor("d34", [128, 4, kle], U16, kind="ExternalInput").ap()
    d36_d = nc.dram_tensor("d36", [128, 2, kle], U16, kind="ExternalInput").ap()
    pso_d = nc.dram_tensor("pso", [64, kle], F32, kind="ExternalOutput").ap()

    Sin = mybir.ActivationFunctionType.Sin
    Alu = mybir.AluOpType
    DR = mybir.MatmulPerfMode.DoubleRow

    ksplits = []
    j = 0
    while j < kle:
        w = min(512, kle - j)
        ksplits.append(slice(j, j + w))
        j += w

    with tile.TileContext(nc) as tc:
        with (
            tc.tile_pool(name="const", bufs=1) as cpool,
            tc.tile_pool(name="qq", bufs=4) as qpool,
            tc.tile_pool(name="acc", bufs=1, space="PSUM") as accp,
            tc.tile_pool(name="ework", bufs=3) as ewp,
            tc.tile_pool(name="resid", bufs=1) as rsp,
        ):
            bias_npi = cpool.tile([128, 1], F32, tag="bnpi")
            nc.vector.memset(bias_npi[:], -PI)
            dd2 = cpool.tile([128, 3, 2, kle], U16, tag="dd2")
            w1 = cpool.tile([128, NCHW, 64], F8, tag="w1")
            w2 = cpool.tile([128, NCHW, 64], F8, tag="w2")

            ps = accp.tile([64, kle], F32, tag="ps")

            # (first_chunk, nchunks, kind, src_group_index)
            GROUPS = ([(0, 2, "seed", None), (36, 1, "d36", None),
                       (34, 2, "d34", None)]
                      + [(2 + 4 * i, 4, "chain", 0 if i == 0 else 2 + i)
                         for i in range(8)])
            NG = len(GROUPS)

            qtiles = {}
            etiles = {}

            def rails(t, j, w):
                return t[:, j::w, :]

            def emit_q(gi):
                c0, nch, kind, src = GROUPS[gi]
                qq = qpool.tile([128, 2 * nch, kle], U16, tag=f"qq{kind}")
                qtiles[gi] = qq
                if kind == "seed":
                    nc.sync.dma_start(qq[:], seed_d[:])
                    return
                if kind == "d36":
                    nc.gpsimd.dma_start(qq[:], d36_d[:])
                    return
                if kind == "d34":
                    nc.gpsimd.dma_start(qq[:], d34_d[:])
                    return
                pq = qtiles[src]
                pn = GROUPS[src][1]
                srcs = [rails(pq, pn - 2, pn), rails(pq, pn - 1, pn)]
                tmps = []
                for j in range(2):
                    t = qpool.tile([128, 2, kle], U16, tag=f"qt{j}")
                    nc.vector.tensor_tensor(
                        t[:], srcs[j], dd2[:, (c0 - 2 + j) % 3, :, :],
                        op=Alu.add)
                    tmps.append(t)
                for j in range(2):
                    nc.vector.tensor_scalar(rails(qq, j, 4), tmps[j][:],
                                            0x7FFF, None,
                                            op0=Alu.bitwise_and)
                tmps2 = []
                for j in range(2):
                    t = qpool.tile([128, 2, kle], U16, tag=f"qu{j}")
                    nc.vector.tensor_tensor(
                        t[:], rails(qq, j, 4), dd2[:, (c0 + j) % 3, :, :],
                        op=Alu.add)
                    tmps2.append(t)
                for j in range(2):
                    nc.vector.tensor_scalar(rails(qq, 2 + j, 4),
                                            tmps2[j][:], 0x7FFF, None,
                                            op0=Alu.bitwise_and)

            def emit_sins(gi):
                c0, nch, kind, src = GROUPS[gi]
                qq = qtiles[gi]
                ee = ewp.tile([128, 2 * nch, kle], F8, tag=f"ee{nch}")
                etiles[gi] = ee
                nc.scalar.activation(ee[:], qq[:], Sin, bias=bias_npi[:],
                                     scale=float(2 * PI / VSCALE))

            def emit_back(gi):
                c0, nch, kind, src = GROUPS[gi]
                ee = etiles.pop(gi)
                if nch == 1:
                    wsl = slice(c0, c0 + 1)
                    for sl in ksplits:
                        nc.tensor.matmul(ps[:, sl], w1[:, wsl, :],
                                         ee[:, 0:1, sl],
                                         start=False, stop=False)
                    for sl in ksplits:
                        nc.tensor.matmul(ps[:, sl], w2[:, wsl, :],
                                         ee[:, 1:2, sl],
                                         start=False, stop=False)
                    return
                for h in range(nch // 2):
                    cc = c0 + 2 * h
                    first = gi == 0 and h == 0
                    last = gi == NG - 1 and h == nch // 2 - 1
                    wsl = slice(cc, cc + 2)
                    for sl in ksplits:
                        nc.tensor.matmul(ps[:, sl], w1[:, wsl, :],
                                         ee[:, 2 * h:2 * h + 2, sl],
                                         start=first, stop=False,
                                         perf_mode=DR)
                    for sl in ksplits:
                        nc.tensor.matmul(ps[:, sl], w2[:, wsl, :],
                                         ee[:, nch + 2 * h:nch + 2 * h + 2, sl],
                                         start=False, stop=last,
                                         perf_mode=DR)

            # direct groups' input DMAs first, then dd2/weights behind them
            emit_q(0)
            emit_q(1)
            emit_q(2)
            nc.sync.dma_start(dd2[:], dd2_d[:])
            nc.gpsimd.dma_start(w1[:], w1_d[:])
            nc.gpsimd.dma_start(w2[:], w2_d[:])

            PF = 1
            for t in range(1, NG + 1 + PF):
                if 3 <= t < NG:
                    emit_q(t)
                if t <= NG:
                    emit_sins(t - 1)
                if t >= 1 + PF:
                    emit_back(t - 1 - PF)
            qtiles.clear()

            psS = rsp.tile([64, kle], F32, tag="psS")
            nc.vector.tensor_scalar(psS[:], ps[:], 0.0, None, op0=Alu.add)
            nc.sync.dma_start(pso_d[:], psS[:])

    nc.compile()
    return nc


_NC_CACHE = {}


def _get_nc(kle):
    if kle not in _NC_CACHE:
        _NC_CACHE[kle] = build_kernel(kle)
    return _NC_CACHE[kle]


def _rep_layout():
    gy_main = np.repeat(np.arange(1, 48), 96)
    gx_main = np.tile(np.arange(-48, 48), 47)
    gy_bnd = np.concatenate([np.full(96, -48), np.arange(-47, 1),
                             np.zeros(48, np.int64)])
    gx_bnd = np.concatenate([np.arange(-48, 48), np.full(48, -48),
                             np.arange(0, 48)])
    gxs = np.concatenate([gx_main, gx_bnd])
    gys = np.concatenate([gy_main, gy_bnd])
    npts = gxs.size
    pad = NCH * 128 - npts
    gxs = np.concatenate([gxs, np.zeros(pad, np.int64)])
    gys = np.concatenate([gys, np.zeros(pad, np.int64)])
    return gxs, gys, npts


def _host_prep(images_reconstructed, kspace_trajectory, kspace_data,
               kspace_mask, sensitivity_maps):
    img = np.asarray(images_reconstructed)
    traj = np.asarray(kspace_trajectory).astype(np.float64)
    kdata = np.asarray(kspace_data)
    mask = np.asarray(kspace_mask).astype(np.float32)
    smaps = np.asarray(sensitivity_maps)
    f8 = ml_dtypes.float8_e4m3

    gxs, gys, npts = _rep_layout()

    x = 0.5 * img[None, ...] * smaps[..., None, None]
    xw = x.reshape(C, N, T).transpose(1, 0, 2).reshape(N, CST)
    rn = (gxs + 48) * 96 + (gys + 48)
    has_m = (gxs >= -47) & (gys >= -47) & ~((gxs == 0) & (gys == 0))
    has_m[npts:] = False
    mn = np.where(has_m, (np.where(has_m, -gxs, 0) + 48) * 96
                  + (np.where(has_m, -gys, 0) + 48), 0)
    xr = xw.real.astype(np.float32)
    xi = xw.imag.astype(np.float32)
    xr_r = xr[rn]
    xi_r = xi[rn]
    xr_r[npts:] = 0.0
    xi_r[npts:] = 0.0
    on = has_m[:, None]
    xr_m = np.where(on, xr[mn], 0.0)
    xi_m = np.where(on, xi[mn], 0.0)
    w1 = np.concatenate([xr_r + xr_m, xi_r + xi_m], 1)
    w2 = -np.concatenate([-(xi_r - xi_m), xr_r - xr_m], 1)
    zpad = np.zeros((128, 64), np.float32)
    w1 = np.ascontiguousarray(np.vstack([w1, zpad]).astype(f8)
                              .reshape(NCHW, 128, 64).transpose(1, 0, 2))
    w2 = np.ascontiguousarray(np.vstack([w2, zpad]).astype(f8)
                              .reshape(NCHW, 128, 64).transpose(1, 0, 2))

    mk = mask.reshape(K)
    act = np.nonzero(mk)[0]
    ke = act.size
    kle = -(-ke // NCORES)
    kle = max(8 * (-(-kle // 8)), 128)
    tx = np.zeros(NCORES * kle)
    ty = np.zeros(NCORES * kle)
    tx[:ke] = traj[0][act]
    ty[:ke] = traj[1][act]
    kd = kdata.reshape(C, K, T).transpose(1, 0, 2).reshape(K, CST)[act]
    mkact = mk[act].astype(np.float64)

    def vq(ph, off):
        v = np.floor((ph + off) * VSCALE + 0.5)
        return np.mod(v, VSCALE).astype(np.uint16)

    gxp = gxs[:128 * 36]
    gyp = gys[:128 * 36]
    dpats = []
    for m3 in range(3):
        nn = np.arange(128 * m3, 128 * m3 + 128)
        dpats.append((gxp[nn + 128] - gxp[nn], gyp[nn + 128] - gyp[nn]))

    in_maps = []
    for i in range(NCORES):
        ksl = slice(i * kle, (i + 1) * kle)
        txc, tyc = tx[ksl], ty[ksl]

        def phase(psl):
            return (gxs[psl, None] * txc[None, :]
                    + gys[psl, None] * tyc[None, :])

        def railpair(c):
            ph = phase(slice(128 * c, 128 * (c + 1)))
            return vq(ph, 0.75), vq(ph, 0.5)

        e0, s0 = railpair(0)
        e1, s1 = railpair(1)
        seed = np.stack([e0, e1, s0, s1], 1)
        e34, s34 = railpair(34)
        e35, s35 = railpair(35)
        d34 = np.stack([e34, e35, s34, s35], 1)
        e36, s36 = railpair(36)
        d36 = np.stack([e36, s36], 1)
        # dd2[m] = (delta[m] + delta[(m+1)%3]) mod 2^15, rail-duplicated
        dd2 = np.empty((128, 3, 2, kle), np.uint16)
        dv = []
        for dgx, dgy in dpats:
            dv.append(vq(dgx[:, None] * txc[None, :]
                         + dgy[:, None] * tyc[None, :], 0.0).astype(np.int64))
        for m in range(3):
            v = (dv[m] + dv[(m + 1) % 3]) & 0x7FFF
            dd2[:, m, 0, :] = v
            dd2[:, m, 1, :] = v
        in_maps.append({
            "w1": w1, "w2": w2,
            "seed": np.ascontiguousarray(seed),
            "dd2": dd2,
            "d34": np.ascontiguousarray(d34),
            "d36": np.ascontiguousarray(d36),
        })
    return in_maps, kd, mkact, ke, kle


def kernel(images_reconstructed, kspace_trajectory, kspace_data,
           kspace_mask, sensitivity_maps, _trace=False):
    in_maps, kd, mkact, ke, kle = _host_prep(
        images_reconstructed, kspace_trajectory, kspace_data,
        kspace_mask, sensitivity_maps)
    nc = _get_nc(kle)
    if _trace:
        import tempfile
        kw = {"tmpdir": tempfile.mkdtemp(prefix="/tmp/bass_trace_")}
    else:
        kw = {}
    res = run_bass_kernel_spmd(nc, in_maps, core_ids=list(range(NCORES)),
                               trace=_trace, **kw)
    pso = np.concatenate([res.results[i]["pso"] for i in range(NCORES)],
                         axis=1)[:, :ke]
    ksp = (pso[:CST] + 1j * pso[CST:]).T.astype(np.complex128)
    d = ksp * mkact[:, None] - kd * mkact[:, None]
    ad = np.abs(d)
    l1, l2 = ad.sum(), (ad * ad).sum()
    a = np.abs(kd * mkact[:, None])
    a1, a2 = a.sum(), (a * a).sum()
    loss = np.asarray(W1 * (l1 / a1) + W2 * math.sqrt(l2) / math.sqrt(a2),
                      dtype=np.float32)
    if _trace:
        return loss, res
    return loss
